# revision 7
# baseline (speedup 1.0000x reference)
"""Multi-head attention (B=4, S=2048, E=512, H=8, D=64) on 8 Trainium2 cores.

Sharding: core c -> (batch b = c//2, head-group g = c%2); each core handles
4 heads of one batch end to end: QKV projections, S x S scores + softmax
(in two layouts: natural [sq, sk] for the attention-probability output,
transposed [sk, sq] recomputed on the PE for the context matmul), the
context, and a partial output projection.  The host pre-transposes/slices
the inputs, sums the two head-group partial outputs per batch, and stacks
the per-core attention blocks into the full [B, H, S, S] tensor.

Per head on device:
  stage2 (T layout): for each sk chunk i: scoresT = kT_h.T @ qT_h (fp32r),
      expT = exp(scoresT), ctx[0:65] += [v_h | 1].T @ expT  (the appended
      ones column makes PSUM row 64 accumulate the softmax denominators).
  ctx normalize: ctx_sb = ctx[0:64] * (1/sums), sums row broadcast across
      partitions via a DRAM bounce.
  stage1 (natural layout): for each sq tile: scores = qT_h.T @ kT_h,
      exp with accumulated row sums, attn = exp * (1/sums), DMA out.
  output projection: out_psum += ctx_sb_h.T @ Wo_h over the 4 heads.
"""

import sys

import numpy as np

if "/opt/trn_rl_repo" not in sys.path:
    sys.path.insert(0, "/opt/trn_rl_repo")

B, S, E, H, D = 4, 2048, 512, 8, 64
HPC = 4          # heads per core
GDIM = HPC * D   # 256 projected columns per core
N_CORES = 8
P = 128
EC = E // P      # 4 contraction chunks

_CACHE = {}


def build_program(s=S, num_devices=N_CORES):
    import concourse.bass as bass
    import concourse.tile as tile
    from concourse import bacc, mybir

    f32 = mybir.dt.float32
    f32r = mybir.dt.float32r
    st = s // P       # row tiles
    nh = s // 1024    # 1024-wide column groups

    nc = bacc.Bacc("TRN2", target_bir_lowering=False, debug=False,
                   num_devices=num_devices)

    xtq = nc.dram_tensor("xtq", [E, s], f32, kind="ExternalInput").ap()
    xtk = nc.dram_tensor("xtk", [E, s], f32, kind="ExternalInput").ap()
    xtv = nc.dram_tensor("xtv", [E, s], f32, kind="ExternalInput").ap()
    wq = nc.dram_tensor("wq", [E, GDIM], f32, kind="ExternalInput").ap()
    wk = nc.dram_tensor("wk", [E, GDIM], f32, kind="ExternalInput").ap()
    wv = nc.dram_tensor("wv", [E, GDIM], f32, kind="ExternalInput").ap()
    wo = nc.dram_tensor("wo", [GDIM, E], f32, kind="ExternalInput").ap()
    bq = nc.dram_tensor("bq", [GDIM], f32, kind="ExternalInput").ap()
    bk = nc.dram_tensor("bk", [GDIM], f32, kind="ExternalInput").ap()
    bvr = nc.dram_tensor("bvr", [HPC * GDIM], f32, kind="ExternalInput").ap()
    attn_out = nc.dram_tensor("attn_part", [HPC, s, s], f32,
                              kind="ExternalOutput").ap()
    outp = nc.dram_tensor("outp", [s, E], f32, kind="ExternalOutput").ap()

    def r(ap):
        return ap.bitcast(f32r)

    with tile.TileContext(nc) as tc:
        with (
            tc.tile_pool(name="singles", bufs=1) as singles,
            tc.tile_pool(name="xt", bufs=2) as xt_pool,
            tc.tile_pool(name="exp", bufs=3) as exp_pool,
            tc.tile_pool(name="attn", bufs=2) as attn_pool,
            tc.tile_pool(name="rows", bufs=2) as rows_pool,
            tc.tile_pool(name="bcast", bufs=2) as bc_pool,
            tc.tile_pool(name="outsb", bufs=2) as out_pool,
            tc.tile_pool(name="small", bufs=8) as small_pool,
            tc.tile_pool(name="ps", bufs=2, space="PSUM") as ps,
            tc.tile_pool(name="dram", bufs=2, space="DRAM") as dram_pool,
        ):
            # ---- persistent SBUF ----
            wq_sb = singles.tile([P, EC, GDIM], f32, tag="wq")
            wk_sb = singles.tile([P, EC, GDIM], f32, tag="wk")
            wv_sb = singles.tile([P, EC, GDIM], f32, tag="wv")
            wo_sb = singles.tile([D, HPC, E], f32, tag="wo")
            bq_sb = singles.tile([P, 2], f32, tag="bq")
            bk_sb = singles.tile([P, 2], f32, tag="bk")
            bv_sb = singles.tile([P, HPC * GDIM], f32, tag="bv")
            # v in natural [sk, d] layout + a per-head ones column feeding the
            # softmax-denominator row of the context matmul
            v_sb = singles.tile([P, st, HPC, D + 1], f32, tag="v")
            qt_sb = [singles.tile([P, s], f32, tag=f"qt{i}", name=f"qt{i}")
                     for i in range(2)]
            kt_sb = [singles.tile([P, s], f32, tag=f"kt{i}", name=f"kt{i}")
                     for i in range(2)]
            ctx_sb = [singles.tile([D, s], f32, tag=f"ctx{i}", name=f"ctx{i}")
                      for i in range(HPC)]

            nc.sync.dma_start(r(wq_sb[:]), r(wq.rearrange("(c p) d -> p c d", p=P)))
            nc.sync.dma_start(r(wk_sb[:]), r(wk.rearrange("(c p) d -> p c d", p=P)))
            nc.sync.dma_start(r(wv_sb[:]), r(wv.rearrange("(c p) d -> p c d", p=P)))
            nc.sync.dma_start(r(wo_sb[:]), r(wo.rearrange("(h d) n -> d h n", d=D)))
            nc.sync.dma_start(bq_sb[:], bq.rearrange("(c p) -> p c", p=P))
            nc.sync.dma_start(bk_sb[:], bk.rearrange("(c p) -> p c", p=P))
            # bvr replicated to every partition (partition-step-0 source AP)
            nc.gpsimd.dma_start(
                out=bv_sb[:],
                in_=bass.AP(tensor=bvr.tensor, offset=bvr.offset,
                            ap=[[0, P], [1, HPC * GDIM]]),
            )
            # codegen rejects a float32r memset; bounce through an f32 tile
            ones_col = singles.tile([P, st * HPC], f32, tag="ones")
            nc.vector.memset(ones_col[:], 1.0)
            nc.vector.tensor_copy(
                r(v_sb[:, :, :, D:D + 1].rearrange("p t h o -> p (t h o)")),
                ones_col[:])

            def psum_1024(j, name):
                # 4 concurrently-live [128, 1024] psum tiles need both tags
                return ps.tile([P, 1024], f32, tag=("sc" if j < 2 else "ctx"),
                               name=name)

            # ---- projections: qT/kT in [d, s] layout (2 heads per tile) ----
            for which, xt_dram, w_sb, dest, b_sb in (
                ("q", xtq, wq_sb, qt_sb, bq_sb),
                ("k", xtk, wk_sb, kt_sb, bk_sb),
            ):
                psums = [psum_1024(j, f"pj{which}{j}") for j in range(2 * nh)]
                for e in range(EC):
                    xt_t = xt_pool.tile([P, s], f32, tag="xt",
                                        name=f"xt{which}{e}")
                    nc.sync.dma_start(r(xt_t[:]), r(xt_dram[e * P:(e + 1) * P, :]))
                    for dt in range(2):
                        for sq in range(2 * nh):
                            nc.tensor.matmul(
                                psums[dt * nh + sq // 2][:, (sq % 2) * 512:
                                                         (sq % 2 + 1) * 512],
                                r(w_sb[:, e, dt * P:(dt + 1) * P]),
                                r(xt_t[:, sq * 512:(sq + 1) * 512]),
                                start=(e == 0), stop=(e == EC - 1),
                            )
                for dt in range(2):
                    for sh in range(nh):
                        nc.vector.tensor_scalar_add(
                            r(dest[dt][:, sh * 1024:(sh + 1) * 1024]),
                            psums[dt * nh + sh][:],
                            b_sb[:, dt:dt + 1],
                        )

            # ---- v projection: natural [sk, d] layout.  Two [128, 256]
            # regions share each PSUM bank, so start=True (which clears the
            # whole bank's has_written bits) is only set on the first matmul
            # touching a bank; later first-writes to the sibling region rely
            # on overwrite-where-unset.  PE executes matmuls in order. ----
            vps = [psum_1024(j, f"pjv{j}") for j in range(st // 4)]
            for e in range(EC):
                xt_t = xt_pool.tile([P, s], f32, tag="xt", name=f"xtv{e}")
                nc.sync.dma_start(r(xt_t[:]), r(xtv[e * P:(e + 1) * P, :]))
                for t in range(st):
                    nc.tensor.matmul(
                        vps[t // 4][:, (t % 4) * GDIM:(t % 4 + 1) * GDIM],
                        r(xt_t[:, t * P:(t + 1) * P]),
                        r(wv_sb[:, e, :]),
                        start=(e == 0 and t % 2 == 0),
                        stop=(e == EC - 1 and t % 2 == 1),
                    )
            for j in range(st // 4):
                nc.vector.tensor_add(
                    r(v_sb[:, j * 4:(j + 1) * 4, :, 0:D]),
                    vps[j][:].rearrange("p (t h d) -> p t h d", h=HPC, d=D),
                    bv_sb[:].rearrange("p (t h d) -> p t h d", h=HPC, d=D),
                )

            # ---- attention, one head at a time ----
            for h in range(HPC):
                dt, sub = h // 2, (h % 2) * D
                qt = qt_sb[dt]
                kt = kt_sb[dt]

                # stage2: transposed scores -> exp -> context (+sums row)
                ctxp = [ps.tile([D + 1, 1024], f32, tag="ctx",
                                name=f"ctxp{h}{j}") for j in range(nh)]
                for i in range(st):
                    for half in range(nh):
                        stt = ps.tile([P, 1024], f32, tag="sc",
                                      name=f"st{h}_{i}_{half}")
                        ee = exp_pool.tile([P, 1024], f32, tag="exp",
                                           name=f"et{h}_{i}_{half}")
                        for j in range(2):
                            sq0 = (half * 2 + j) * 512
                            nc.tensor.matmul(
                                stt[:, j * 512:(j + 1) * 512],
                                r(kt[sub:sub + D, i * P:(i + 1) * P]),
                                r(qt[sub:sub + D, sq0:sq0 + 512]),
                            )
                        nc.scalar.activation(
                            r(ee[:]), stt[:],
                            mybir.ActivationFunctionType.Exp)
                        for j in range(2):
                            nc.tensor.matmul(
                                ctxp[half][:, j * 512:(j + 1) * 512],
                                r(v_sb[:, i, h, :]),
                                r(ee[:, j * 512:(j + 1) * 512]),
                                start=(i == 0), stop=(i == st - 1),
                            )

                # softmax sums row -> DRAM bounce -> partition broadcast
                row_sb = rows_pool.tile([D + 1, s], f32, tag="row",
                                        name=f"row{h}")
                for half in range(nh):
                    nc.vector.tensor_copy(
                        row_sb[D:D + 1, half * 1024:(half + 1) * 1024],
                        ctxp[half][D:D + 1, :])
                row_dram = dram_pool.tile([s], f32, tag="rowd",
                                          name=f"rowd{h}")
                nc.sync.dma_start(row_dram[:], row_sb[D:D + 1, :])
                sums_bc = bc_pool.tile([D, s], f32, tag="bc", name=f"sbc{h}")
                nc.gpsimd.dma_start(
                    out=sums_bc[:],
                    in_=bass.AP(tensor=row_dram.tensor, offset=row_dram.offset,
                                ap=[[0, D], [1, s]]),
                )
                rec_bc = bc_pool.tile([D, s], f32, tag="bc", name=f"rbc{h}")
                nc.vector.reciprocal(rec_bc[:], sums_bc[:])
                for half in range(nh):
                    nc.vector.tensor_mul(
                        r(ctx_sb[h][:, half * 1024:(half + 1) * 1024]),
                        ctxp[half][0:D, :],
                        rec_bc[:, half * 1024:(half + 1) * 1024])

                # stage1: natural-layout scores -> normalized attention out
                for t in range(st):
                    attn_t = attn_pool.tile([P, s], f32, tag="attn",
                                            name=f"at{h}_{t}")
                    acc = small_pool.tile([P, 2], f32, tag="acc",
                                          name=f"acc{h}_{t}")
                    tot = small_pool.tile([P, 2], f32, tag="tot",
                                          name=f"tot{h}_{t}")
                    e1s = []
                    for half in range(nh):
                        s1 = ps.tile([P, 1024], f32, tag="sc",
                                     name=f"s1_{h}_{t}_{half}")
                        e1 = exp_pool.tile([P, 1024], f32, tag="exp1",
                                           bufs=4, name=f"e1_{h}_{t}_{half}")
                        for j in range(2):
                            sk0 = (half * 2 + j) * 512
                            nc.tensor.matmul(
                                s1[:, j * 512:(j + 1) * 512],
                                r(qt[sub:sub + D, t * P:(t + 1) * P]),
                                r(kt[sub:sub + D, sk0:sk0 + 512]),
                            )
                        nc.scalar.activation(
                            e1[:], s1[:], mybir.ActivationFunctionType.Exp,
                            accum_out=acc[:, half:half + 1])
                        e1s.append(e1)
                    if nh == 2:
                        nc.vector.tensor_add(tot[:, 0:1], acc[:, 0:1],
                                             acc[:, 1:2])
                    else:
                        nc.vector.tensor_copy(tot[:, 0:1], acc[:, 0:1])
                    nc.vector.reciprocal(tot[:, 1:2], tot[:, 0:1])
                    for half in range(nh):
                        nc.vector.tensor_scalar_mul(
                            attn_t[:, half * 1024:(half + 1) * 1024],
                            e1s[half][:], tot[:, 1:2])
                    nc.sync.dma_start(
                        attn_out[h, t * P:(t + 1) * P, :], attn_t[:])

            # ---- output projection ----
            for tp in range(st // 2):
                po = ps.tile([P, 1024], f32, tag="sc", name=f"po{tp}")
                for half in range(2):
                    t = tp * 2 + half
                    for h in range(HPC):
                        nc.tensor.matmul(
                            po[:, half * 512:(half + 1) * 512],
                            r(ctx_sb[h][:, t * P:(t + 1) * P]),
                            r(wo_sb[:, h, :]),
                            start=(h == 0), stop=(h == HPC - 1),
                        )
                ob = out_pool.tile([P, 1024], f32, tag="ob", name=f"ob{tp}")
                nc.vector.tensor_copy(ob[:], po[:])
                nc.sync.dma_start(
                    outp.rearrange("(tp half p) n -> p tp half n",
                                   p=P, half=2)[:, tp, :, :],
                    ob[:].rearrange("p (half n) -> p half n", half=2))

    nc.compile()
    return nc


def make_in_maps(query, key, value, Wq, bq, Wk, bk, Wv, bv, Wo):
    scale = np.float32(1.0 / np.sqrt(np.float32(D)))
    in_maps = []
    for c in range(N_CORES):
        b, g = c // 2, c % 2
        cols = slice(g * GDIM, (g + 1) * GDIM)
        in_maps.append({
            "xtq": np.ascontiguousarray(query[b].T),
            "xtk": np.ascontiguousarray(key[b].T),
            "xtv": np.ascontiguousarray(value[b].T),
            "wq": np.ascontiguousarray(Wq[:, cols]) * scale,
            "wk": np.ascontiguousarray(Wk[:, cols]),
            "wv": np.ascontiguousarray(Wv[:, cols]),
            "wo": np.ascontiguousarray(Wo[cols, :]),
            "bq": np.ascontiguousarray(bq[cols]) * scale,
            "bk": np.ascontiguousarray(bk[cols]),
            "bvr": np.tile(np.ascontiguousarray(bv[cols]), HPC),
        })
    return in_maps


def assemble(results, bo):
    attn = np.empty((B, H, S, S), np.float32)
    output = np.empty((B, S, E), np.float32)
    for c in range(N_CORES):
        b, g = c // 2, c % 2
        attn[b, g * HPC:(g + 1) * HPC] = results[c]["attn_part"]
    for b in range(B):
        output[b] = results[2 * b]["outp"] + results[2 * b + 1]["outp"] + bo
    return output, attn


def kernel(query, key, value, Wq, bq, Wk, bk, Wv, bv, Wo, bo):
    from concourse.bass_utils import run_bass_kernel_spmd

    if "nc" not in _CACHE:
        _CACHE["nc"] = build_program()
    nc = _CACHE["nc"]

    arrs = [np.asarray(a, np.float32) for a in
            (query, key, value, Wq, bq, Wk, bk, Wv, bv, Wo, bo)]
    query, key, value, Wq, bq, Wk, bk, Wv, bv, Wo, bo = arrs
    in_maps = make_in_maps(query, key, value, Wq, bq, Wk, bk, Wv, bv, Wo)
    res = run_bass_kernel_spmd(nc, in_maps, list(range(N_CORES)))
    return assemble(res.results, bo)
